# revision 25
# baseline (speedup 1.0000x reference)
"""Trainium2 Bass kernel for nn_Box3DHeuristic (box-3D flavor transformation
heuristic).

Math (per sample b, D=800 quadrature directions):
  4 species moments (N, F) -> fluxfac -> Z (closure) -> maxwellian-ish
  angular distributions g_s(d) = exp(Vaug(d) . Aaug_s(b)) with
  Aaug = [Z*Fx/|F|, Z*Fy/|F|, Z*Fz/|F|, ln(N/(4 pi) * z/sinh z)],
  delta = g0-g1-g2+g3, masked/weighted angular sums -> coeffs -> mixed
  moments + growth rate.

Layout: directions on SBUF partitions (7 chunks of 128, padded 800->896),
samples on the free dim (tiles of 512). Per (chunk, tile):
  PE : 4 arg matmuls (fp32r, K=4) + 10 bf16 reduction matmuls
       (Stot/Spos per species via tile_position col-groups, Ip/Im)
  ACT: 4 exp (psum -> bf16 sbuf) + relu(delta)
  DVE: relu(-delta), 4x fused (delta>=0)*g
  POOL: the 3-op delta chain
Reductions accumulate in PSUM over the 7 chunks; accumulators are DMA'd to
DRAM scratch and gathered back transposed (samples on partitions) for the
scalar postprocессing, which is done batched for the whole core.

8 cores, pure data parallel over the 32768-sample batch.
"""

import math
import sys

sys.path.insert(0, "/opt/trn_rl_repo")

import numpy as np
import ml_dtypes

import concourse.bacc as bacc
import concourse.mybir as mybir
from concourse import tile
from concourse.bass_utils import run_bass_kernel_spmd

F32 = mybir.dt.float32
F32R = mybir.dt.float32r
BF16 = mybir.dt.bfloat16
AF = mybir.ActivationFunctionType
ALU = mybir.AluOpType

B_TOTAL = 32768
N_CORES = 8
NPC = B_TOTAL // N_CORES  # samples per core = 4096
NB = 512  # samples per tile
D = 800
DP = 896  # padded dirs
NCH = DP // 128  # 7 chunks
EPS = 1e-12
INV_4PI = 1.0 / (4.0 * math.pi)
LN2 = math.log(2.0)


def _build(npc=NPC):
    """Build the per-core Bass program (SPMD across 8 cores)."""
    nt = npc // NB  # sample tiles
    npb = npc // 128  # partition blocks of samples
    pb_per_tile = NB // 128  # 4

    nc = bacc.Bacc("TRN2", target_bir_lowering=False, debug=False)

    f4 = nc.declare_dram_parameter("f4", [npc, 24], F32, isOutput=False)
    # block-diagonal Vaug: row s*4+k of the s-block is Vaug[k], else 0 --
    # lets the arg matmul consume the full [16, NB] transposed-A tile
    # without 4-partition slicing (engines need 32-aligned bases)
    vaugbd = nc.declare_dram_parameter("vaugbd", [16, 4 * DP], F32R, isOutput=False)
    wq = nc.declare_dram_parameter("wq", [128, 4 * NCH], BF16, isOutput=False)
    ident = nc.declare_dram_parameter("ident", [128, 128], F32, isOutput=False)
    outf4 = nc.declare_dram_parameter("outf4", [npc, 24], F32, isOutput=True)
    growth = nc.declare_dram_parameter("growth", [npc], F32, isOutput=True)

    # DRAM scratch for the accumulator re-layout (transpose via DMA),
    # rows = 4s+q packed
    stot_d = nc.dram_tensor("stot_d", [16, npc], F32)
    spos_d = nc.dram_tensor("spos_d", [16, npc], F32)
    ipim_d = nc.dram_tensor("ipim_d", [2, npc], F32)

    with tile.TileContext(nc) as tc:
        with (
            tc.tile_pool(name="cst", bufs=1) as cst,
            tc.tile_pool(name="pre", bufs=1) as pre,
            tc.tile_pool(name="sb", bufs=2) as sb,
            tc.tile_pool(name="gp", bufs=2) as gp,
            tc.tile_pool(name="post", bufs=1) as post,
            tc.tile_pool(name="ps_work", bufs=3, space="PSUM") as ps_work,
            tc.tile_pool(name="ps_rhs", bufs=2, space="PSUM") as ps_rhs_pool,
            tc.tile_pool(name="ps_acc", bufs=1, space="PSUM") as ps_acc,
        ):
            # ---- constants in ----
            v_sb = cst.tile([16, 4 * DP], F32R, tag="v")
            nc.sync.dma_start(v_sb[:], vaugbd[:])
            wq_sb = cst.tile([128, 4 * NCH], BF16, tag="wq")
            nc.sync.dma_start(wq_sb[:], wq[:])
            id_sb = cst.tile([128, 128], F32, tag="id")
            nc.sync.dma_start(id_sb[:], ident[:])

            # ---- phase 1: load + preprocess (whole core, batched) ----
            raw = pre.tile([128, npb * 24], F32, tag="raw")
            r3 = raw[:].rearrange("p (t c) -> p t c", c=24)
            nc.sync.dma_start(
                r3, f4[:].rearrange("(t p) c -> p t c", p=128)
            )
            tc.strict_bb_all_engine_barrier()

            # species moments  spec[p, t, s, c], c = [Fx, Fy, Fz, N]
            spec = pre.tile([128, npb * 16], F32, tag="spec")
            s4 = spec[:].rearrange("p (t s c) -> p t s c", s=4, c=4)
            nc.vector.tensor_copy(s4[:, :, 0, :], r3[:, :, 0:4])
            nc.vector.tensor_copy(s4[:, :, 1, :], r3[:, :, 12:16])
            nc.vector.tensor_add(s4[:, :, 2, :], r3[:, :, 4:8], r3[:, :, 8:12])
            nc.vector.tensor_add(s4[:, :, 3, :], r3[:, :, 16:20], r3[:, :, 20:24])

            def qt(tag):
                t = pre.tile([128, npb * 4], F32, tag=tag)
                return t, t[:].rearrange("p (t s) -> p t s", s=4)

            sq = pre.tile([128, npb * 16], F32, tag="sq")
            q4 = sq[:].rearrange("p (t s c) -> p t s c", s=4, c=4)
            nc.vector.tensor_mul(sq[:], spec[:], spec[:])
            n2_t, n2 = qt("n2")
            nc.vector.tensor_add(n2, q4[:, :, :, 0], q4[:, :, :, 1])
            nc.vector.tensor_add(n2, n2, q4[:, :, :, 2])
            nc.vector.tensor_scalar(n2, n2, 1e-24, None, ALU.max)
            lnn2_t, lnn2 = qt("lnn2")
            nc.scalar.activation(lnn2, n2, AF.Ln)
            nrm_t, nrm = qt("nrm")
            nc.scalar.activation(nrm, lnn2, AF.Exp, scale=0.5)
            rn_t, rn = qt("rn")
            nc.scalar.activation(rn, lnn2, AF.Exp, scale=-0.5)
            recn_t, recn = qt("recn")
            nc.vector.reciprocal(recn, s4[:, :, :, 3])
            ff_t, ff = qt("ff")
            nc.vector.tensor_mul(ff, nrm, recn)
            nc.vector.tensor_scalar(ff, ff, 0.999999, None, ALU.min)

            # Z = 2f / max(2*(1-f)*(1+1.01524 f)/den, 1e-6)
            f2_t, f2 = qt("f2")
            nc.vector.tensor_mul(f2, ff, ff)
            f4q_t, f4q = qt("f4q")
            nc.vector.tensor_mul(f4q, f2, f2)
            f6_t, f6 = qt("f6")
            nc.vector.tensor_mul(f6, f4q, f2)
            f8_t, f8 = qt("f8")
            nc.vector.tensor_mul(f8, f4q, f4q)
            den_t, den = qt("den")
            nc.vector.tensor_scalar(den, f2, -1.00651, 3.0, ALU.mult, ALU.add)
            nc.vector.scalar_tensor_tensor(den, f4q, -0.962251, den, ALU.mult, ALU.add)
            nc.vector.scalar_tensor_tensor(den, f6, 1.47353, den, ALU.mult, ALU.add)
            nc.vector.scalar_tensor_tensor(den, f8, -0.48953, den, ALU.mult, ALU.add)
            u_t, uu = qt("u")
            nc.vector.tensor_scalar(uu, ff, 1.01524, 1.0, ALU.mult, ALU.add)
            v_t, vv = qt("vv")
            nc.vector.tensor_scalar(vv, ff, -1.0, 1.0, ALU.mult, ALU.add)
            uv_t, uv = qt("uv")
            nc.vector.tensor_mul(uv, uu, vv)
            rden_t, rden = qt("rden")
            nc.vector.reciprocal(rden, den)
            tq_t, tq = qt("tq")
            nc.vector.tensor_mul(tq, uv, rden)
            zden_t, zden = qt("zden")
            nc.vector.tensor_scalar(zden, tq, 2.0, 1e-6, ALU.mult, ALU.max)
            rzden_t, rzden = qt("rzden")
            nc.vector.reciprocal(rzden, zden)
            zz_t, zz = qt("zz")
            nc.vector.scalar_tensor_tensor(zz, ff, 2.0, rzden, ALU.mult, ALU.mult)

            # w = z/sinh(z) = 2 z e^-z / (1 - e^-2z), series for z < 1e-3
            em_t, em = qt("em")
            nc.scalar.activation(em, zz, AF.Exp, scale=-1.0)
            em2_t, em2 = qt("em2")
            nc.vector.tensor_mul(em2, em, em)
            d2_t, d2 = qt("d2")
            nc.vector.tensor_scalar(d2, em2, -1.0, 1.0, ALU.mult, ALU.add)
            nc.vector.tensor_scalar(d2, d2, 1e-20, None, ALU.max)
            rd2_t, rd2 = qt("rd2")
            nc.vector.reciprocal(rd2, d2)
            num_t, num = qt("num")
            nc.vector.scalar_tensor_tensor(num, zz, 2.0, em, ALU.mult, ALU.mult)
            wzs_t, wzs = qt("wzs")
            nc.vector.tensor_mul(wzs, num, rd2)
            z2_t, z2 = qt("z2")
            nc.vector.tensor_mul(z2, zz, zz)
            z4_t, z4 = qt("z4")
            nc.vector.tensor_mul(z4, z2, z2)
            ser_t, ser = qt("ser")
            nc.vector.tensor_scalar(ser, z2, -1.0 / 6.0, 1.0, ALU.mult, ALU.add)
            nc.vector.scalar_tensor_tensor(ser, z4, 1.0 / 120.0, ser, ALU.mult, ALU.add)
            msmall_t = pre.tile([128, npb * 4], mybir.dt.uint8, tag="msmall")
            msmall = msmall_t[:].rearrange("p (t s) -> p t s", s=4)
            nc.vector.tensor_single_scalar(msmall, zz, 1e-3, ALU.is_lt)
            nc.vector.copy_predicated(wzs, msmall, ser)

            # lnc = ln(N * wzs * INV_4PI); x/xbar species carry -ln2 (their
            # N/F are sums of 2 flavors, not means -- scale cancels
            # everywhere except here)
            prod_t, prod = qt("prod")
            nc.vector.tensor_mul(prod, s4[:, :, :, 3], wzs)
            lnc_t, lnc = qt("lnc")
            nc.scalar.activation(lnc, prod, AF.Ln, scale=INV_4PI)
            nc.vector.tensor_scalar(
                lnc[:, :, 2:4], lnc[:, :, 2:4], -LN2, None, ALU.add
            )

            # pre_A[p, t, s, k] = [Z*Fx/|F|, Z*Fy/|F|, Z*Fz/|F|, lnc]
            zr_t, zr = qt("zr")
            nc.vector.tensor_mul(zr, zz, rn)
            pre_a = pre.tile([128, npb * 16], F32, tag="pre_a")
            a4 = pre_a[:].rearrange("p (t s k) -> p t s k", s=4, k=4)
            for k in range(3):
                nc.vector.tensor_mul(a4[:, :, :, k], s4[:, :, :, k], zr)
            nc.vector.tensor_copy(a4[:, :, :, 3], lnc)

            # ---- phase 2: per sample-tile main loop ----
            for n in range(nt):
                # transpose pre_A block -> rhs [16, 512] (k,s on partitions)
                ps_rhs = ps_rhs_pool.tile([16, NB], F32, tag="ps_rhs")
                for j in range(pb_per_tile):
                    pb = n * pb_per_tile + j
                    nc.tensor.transpose(
                        ps_rhs[:, 128 * j : 128 * (j + 1)],
                        pre_a[:, 16 * pb : 16 * (pb + 1)],
                        id_sb[:],
                    )
                rhs_all = sb.tile([16, NB], F32R, tag="rhs")
                nc.vector.tensor_copy(rhs_all[:], ps_rhs[:])

                acc_tot = ps_acc.tile([128, NB], F32, tag="acc_tot")
                acc_pos = ps_acc.tile([128, NB], F32, tag="acc_pos")
                acc_ipim = ps_acc.tile([128, NB], F32, tag="acc_ipim")

                for c in range(NCH):
                    gs = []
                    for s in range(4):
                        arg = ps_work.tile([128, NB], F32, tag="arg")
                        nc.tensor.matmul(
                            arg[:],
                            v_sb[:, s * DP + 128 * c : s * DP + 128 * (c + 1)],
                            rhs_all[:],
                            start=True,
                            stop=True,
                        )
                        g = gp.tile([128, NB], BF16, tag=f"g{s}")
                        nc.scalar.activation(g[:], arg[:], AF.Exp)
                        gs.append(g)

                    t1 = gp.tile([128, NB], BF16, tag="t1")
                    nc.gpsimd.tensor_sub(t1[:], gs[0][:], gs[1][:])
                    t2 = gp.tile([128, NB], BF16, tag="t2")
                    nc.gpsimd.tensor_sub(t2[:], gs[2][:], gs[3][:])
                    delta = gp.tile([128, NB], BF16, tag="delta")
                    nc.gpsimd.tensor_sub(delta[:], t1[:], t2[:])

                    rpos = gp.tile([128, NB], BF16, tag="rpos")
                    nc.scalar.activation(rpos[:], delta[:], AF.Relu)
                    rneg = gp.tile([128, NB], BF16, tag="rneg")
                    nc.vector.tensor_scalar(
                        rneg[:], delta[:], -1.0, 0.0, ALU.mult, ALU.max
                    )

                    wqc = wq_sb[:, 4 * c : 4 * c + 4]
                    mmkw = dict(start=(c == 0), stop=(c == NCH - 1))
                    for s in range(4):
                        gpos = gp.tile([128, NB], BF16, tag=f"gp{s}")
                        nc.vector.scalar_tensor_tensor(
                            gpos[:], delta[:], 0.0, gs[s][:], ALU.is_ge, ALU.mult
                        )
                        nc.tensor.matmul(
                            acc_tot[32 * s : 32 * s + 4, :],
                            wqc,
                            gs[s][:],
                            tile_position=(0, 32 * s),
                            **mmkw,
                        )
                        nc.tensor.matmul(
                            acc_pos[32 * s : 32 * s + 4, :],
                            wqc,
                            gpos[:],
                            tile_position=(0, 32 * s),
                            **mmkw,
                        )
                    nc.tensor.matmul(
                        acc_ipim[0:4, :], wqc, rpos[:], tile_position=(0, 0), **mmkw
                    )
                    nc.tensor.matmul(
                        acc_ipim[32:36, :], wqc, rneg[:], tile_position=(0, 32), **mmkw
                    )

                # PSUM has no DMA route: stage written row-groups through
                # SBUF (engine partition bases must be 32-aligned; the DMAs
                # do the row packing)
                sl = slice(NB * n, NB * (n + 1))
                sb_tot = sb.tile([128, NB], F32, tag="sb_tot")
                sb_pos = sb.tile([128, NB], F32, tag="sb_pos")
                sb_ipim = sb.tile([36, NB], F32, tag="sb_ipim")
                for s in range(4):
                    nc.vector.tensor_copy(
                        sb_tot[32 * s : 32 * s + 4, :], acc_tot[32 * s : 32 * s + 4, :]
                    )
                    nc.scalar.copy(
                        sb_pos[32 * s : 32 * s + 4, :], acc_pos[32 * s : 32 * s + 4, :]
                    )
                    nc.sync.dma_start(
                        stot_d[4 * s : 4 * s + 4, sl], sb_tot[32 * s : 32 * s + 4, :]
                    )
                    nc.sync.dma_start(
                        spos_d[4 * s : 4 * s + 4, sl], sb_pos[32 * s : 32 * s + 4, :]
                    )
                nc.vector.tensor_copy(sb_ipim[0:4, :], acc_ipim[0:4, :])
                nc.vector.tensor_copy(sb_ipim[32:36, :], acc_ipim[32:36, :])
                nc.sync.dma_start(ipim_d[0:1, sl], sb_ipim[3:4, :])
                nc.sync.dma_start(ipim_d[1:2, sl], sb_ipim[35:36, :])

            # ---- phase 3: reload scratch, PE-transpose to sample-major ----
            ld_tot = post.tile([16, npc], F32, tag="ld_tot")
            nc.sync.dma_start(ld_tot[:], stot_d[:])
            ld_pos = post.tile([16, npc], F32, tag="ld_pos")
            nc.sync.dma_start(ld_pos[:], spos_d[:])
            ld_ipim = post.tile([2, npc], F32, tag="ld_ipim")
            nc.sync.dma_start(ld_ipim[:], ipim_d[:])
            tc.strict_bb_all_engine_barrier()

            # psum tiles (reuse the accumulator slots, freed by now)
            ptot_ps = ps_acc.tile([128, NB], F32, tag="acc_tot")
            ppos_ps = ps_acc.tile([128, NB], F32, tag="acc_pos")
            pipim_ps = ps_acc.tile([128, NB], F32, tag="acc_ipim")
            for t in range(npb):
                cs = slice(128 * t, 128 * (t + 1))
                nc.tensor.transpose(
                    ptot_ps[:, 16 * t : 16 * (t + 1)], ld_tot[:, cs], id_sb[0:16, 0:16]
                )
                nc.tensor.transpose(
                    ppos_ps[:, 16 * t : 16 * (t + 1)], ld_pos[:, cs], id_sb[0:16, 0:16]
                )
                nc.tensor.transpose(
                    pipim_ps[:, 2 * t : 2 * (t + 1)], ld_ipim[:, cs], id_sb[0:2, 0:2]
                )

            ptot_sb = post.tile([128, npb * 16], F32, tag="ptot_sb")
            nc.vector.tensor_copy(ptot_sb[:], ptot_ps[:, 0 : npb * 16])
            ppos_sb = post.tile([128, npb * 16], F32, tag="ppos_sb")
            nc.scalar.copy(ppos_sb[:], ppos_ps[:, 0 : npb * 16])
            pipim_sb = post.tile([128, npb * 2], F32, tag="pipim_sb")
            nc.vector.tensor_copy(pipim_sb[:], pipim_ps[:, 0 : npb * 2])
            ptot_t = ptot_sb[:]
            ptot = ptot_t.rearrange("p (t s q) -> p t s q", s=4, q=4)
            ppos_t = ppos_sb[:]
            ip2 = pipim_sb[:].rearrange("p (t i) -> p t i", i=2)

            def pt(tag, w=1):
                t = post.tile([128, npb * w], F32, tag=tag)
                if w == 1:
                    return t, t[:]
                return t, t[:].rearrange("p (t s) -> p t s", s=w)

            ipt, ip = pt("ip")
            nc.vector.tensor_scalar(ip, ip2[:, :, 0], 0.0, None, ALU.max)
            imt, im = pt("im")
            nc.vector.tensor_scalar(im, ip2[:, :, 1], 0.0, None, ALU.max)
            # growth = sqrt(ip*im) via ln/exp
            gprod_t, gprod = pt("gprod")
            nc.vector.tensor_mul(gprod, ip, im)
            nc.vector.tensor_scalar(gprod, gprod, 1e-38, None, ALU.max)
            grw_t, grw = pt("grw")
            nc.scalar.activation(grw, gprod, AF.Ln)
            nc.scalar.activation(grw, grw, AF.Exp, scale=0.5)
            # growth is sample-contiguous in DRAM: PE-transpose first
            grw_ps = ps_work.tile([npb, 128], F32, tag="arg")
            nc.tensor.transpose(grw_ps[:], grw, id_sb[:])
            grw_sb = post.tile([npb, 128], F32, tag="grw_sb")
            nc.vector.tensor_copy(grw_sb[:], grw_ps[:])
            nc.sync.dma_start(
                growth[:].rearrange("(t p) -> t p", p=128), grw_sb[:]
            )

            ips_t, ips = pt("ips")
            nc.vector.tensor_scalar(ips, ip, EPS, None, ALU.max)
            ims_t, ims = pt("ims")
            nc.vector.tensor_scalar(ims, im, EPS, None, ALU.max)
            rip_t, rip = pt("rip")
            nc.vector.reciprocal(rip, ips)
            rim_t, rim = pt("rim")
            nc.vector.reciprocal(rim, ims)
            cp_t, cp = pt("cp")
            nc.vector.scalar_tensor_tensor(cp, ims, -2.0 / 3.0, rip, ALU.mult, ALU.mult)
            nc.vector.tensor_scalar(cp, cp, 1.0, None, ALU.add)
            cn_t, cn = pt("cn")
            nc.vector.scalar_tensor_tensor(cn, ips, -2.0 / 3.0, rim, ALU.mult, ALU.mult)
            nc.vector.tensor_scalar(cn, cn, 1.0, None, ALU.add)
            third_t, third = pt("third")
            nc.vector.memset(third, 1.0 / 3.0)
            clt_t = post.tile([128, npb], mybir.dt.uint8, tag="clt")
            clt = clt_t[:]
            nc.vector.tensor_tensor(clt, ip, im, ALU.is_lt)
            cge_t = post.tile([128, npb], mybir.dt.uint8, tag="cge")
            cge = cge_t[:]
            nc.vector.tensor_tensor(cge, ip, im, ALU.is_ge)
            nc.vector.copy_predicated(cp, clt, third)
            nc.vector.copy_predicated(cn, cge, third)
            nc.vector.tensor_scalar(cp, cp, 0.0, 1.0, ALU.max, ALU.min)
            nc.vector.tensor_scalar(cn, cn, 0.0, 1.0, ALU.max, ALU.min)
            # half coeffs for the assembly
            cnh_t, cnh = pt("cnh")
            nc.vector.tensor_scalar(cnh, cn, 0.5, None, ALU.mult)
            dch_t, dch = pt("dch")
            nc.vector.tensor_sub(dch, cp, cn)
            nc.vector.tensor_scalar(dch, dch, 0.5, None, ALU.mult)

            # Uh[p,t,s,q] = 0.5 * (cn*Stot + (cp-cn)*Spos)
            uh = post.tile([128, npb * 16], F32, tag="uh")
            u4 = uh[:].rearrange("p (t s q) -> p t s q", s=4, q=4)
            tmp16 = post.tile([128, 16], F32, tag="tmp16")
            for t in range(npb):
                nc.vector.tensor_scalar(
                    tmp16[:], ppos_t[:, 16 * t : 16 * (t + 1)],
                    dch_t[:, t : t + 1], None, ALU.mult,
                )
                nc.vector.scalar_tensor_tensor(
                    uh[:, 16 * t : 16 * (t + 1)],
                    ptot_t[:, 16 * t : 16 * (t + 1)],
                    cnh_t[:, t : t + 1],
                    tmp16[:],
                    ALU.mult,
                    ALU.add,
                )

            # output species sums:
            # out0 = 2*Uh0 + S2 - 2*Uh2        (nue_t)
            # out1 = 2*Uh1 + S3 - 2*Uh3        (nuebar_t)
            # out2 = 0.5*(S0+S2) + Uh2 - Uh0   (nux_t)
            # out3 = 0.5*(S1+S3) + Uh3 - Uh1   (nuxbar_t)
            newq = post.tile([128, npb * 16], F32, tag="newq")
            n4 = newq[:].rearrange("p (t s q) -> p t s q", s=4, q=4)
            ta = post.tile([128, npb * 4], F32, tag="ta")
            ta3 = ta[:].rearrange("p (t q) -> p t q", q=4)
            tb = post.tile([128, npb * 4], F32, tag="tb")
            tb3 = tb[:].rearrange("p (t q) -> p t q", q=4)
            nc.vector.scalar_tensor_tensor(
                ta3, u4[:, :, 2, :], -2.0, ptot[:, :, 2, :], ALU.mult, ALU.add
            )
            nc.vector.scalar_tensor_tensor(
                n4[:, :, 0, :], u4[:, :, 0, :], 2.0, ta3, ALU.mult, ALU.add
            )
            nc.vector.scalar_tensor_tensor(
                ta3, u4[:, :, 3, :], -2.0, ptot[:, :, 3, :], ALU.mult, ALU.add
            )
            nc.vector.scalar_tensor_tensor(
                n4[:, :, 1, :], u4[:, :, 1, :], 2.0, ta3, ALU.mult, ALU.add
            )
            nc.vector.tensor_add(ta3, ptot[:, :, 0, :], ptot[:, :, 2, :])
            nc.vector.tensor_sub(tb3, u4[:, :, 2, :], u4[:, :, 0, :])
            nc.vector.scalar_tensor_tensor(
                n4[:, :, 2, :], ta3, 0.5, tb3, ALU.mult, ALU.add
            )
            nc.vector.tensor_add(ta3, ptot[:, :, 1, :], ptot[:, :, 3, :])
            nc.vector.tensor_sub(tb3, u4[:, :, 3, :], u4[:, :, 1, :])
            nc.vector.scalar_tensor_tensor(
                n4[:, :, 3, :], ta3, 0.5, tb3, ALU.mult, ALU.add
            )

            # assemble [p, t, 6 slots, 4] and store
            osb = post.tile([128, npb * 24], F32, tag="osb")
            o4 = osb[:].rearrange("p (t s c) -> p t s c", s=6, c=4)
            for slot, src in ((0, 0), (1, 2), (2, 2), (3, 1), (4, 3), (5, 3)):
                nc.vector.tensor_copy(o4[:, :, slot, :], n4[:, :, src, :])
            nc.sync.dma_start(
                outf4[:].rearrange("(t p) c -> p t c", p=128),
                osb[:].rearrange("p (t c) -> p t c", c=24),
            )

    nc.compile()
    return nc


def _make_consts(dir_x, dir_y, dir_z, quad_w):
    """Build block-diag Vaug [16, 4*DP] (f32) and wq [128, 4*NCH] (bf16)."""
    vx = np.zeros(DP, np.float32)
    vy = np.zeros(DP, np.float32)
    vz = np.zeros(DP, np.float32)
    w = np.zeros(DP, np.float32)
    vx[:D], vy[:D], vz[:D], w[:D] = dir_x, dir_y, dir_z, quad_w
    vaug = np.stack([vx, vy, vz, np.ones(DP, np.float32)])  # [4, DP]
    vaugbd = np.zeros((16, 4 * DP), np.float32)
    for s in range(4):
        vaugbd[4 * s : 4 * s + 4, s * DP : (s + 1) * DP] = vaug
    # wq[p, 4c+j]: dir d = 128c + p, cols [w*vx, w*vy, w*vz, w]
    cols = np.stack([w * vx, w * vy, w * vz, w], axis=1)  # [DP, 4]
    wq = cols.reshape(NCH, 128, 4).transpose(1, 0, 2).reshape(128, 4 * NCH)
    return vaugbd, wq.astype(ml_dtypes.bfloat16)


_NC_CACHE = {}
TRACE = False  # set True (e.g. from test.py) to capture an NTFF profile
LAST_RESULTS = None  # BassKernelResults of the most recent kernel() call


def kernel(F4_in, dir_x, dir_y, dir_z, quad_w):
    F4_in = np.asarray(F4_in, np.float32)
    dir_x = np.asarray(dir_x, np.float32)
    dir_y = np.asarray(dir_y, np.float32)
    dir_z = np.asarray(dir_z, np.float32)
    quad_w = np.asarray(quad_w, np.float32)
    b = F4_in.shape[0]
    npc = b // N_CORES
    if npc not in _NC_CACHE:
        _NC_CACHE[npc] = _build(npc)
    nc = _NC_CACHE[npc]

    vaugbd, wq = _make_consts(dir_x, dir_y, dir_z, quad_w)
    ident = np.eye(128, dtype=np.float32)
    flat = np.ascontiguousarray(F4_in.reshape(b, 24))
    in_maps = [
        dict(
            f4=flat[i * npc : (i + 1) * npc],
            vaugbd=vaugbd,
            wq=wq,
            ident=ident,
        )
        for i in range(N_CORES)
    ]
    res = run_bass_kernel_spmd(
        nc, in_maps, core_ids=list(range(N_CORES)), trace=TRACE
    )
    global LAST_RESULTS
    LAST_RESULTS = res
    f4mix = np.concatenate([r["outf4"] for r in res.results]).reshape(b, 2, 3, 4)
    grw = np.concatenate([r["growth"] for r in res.results])
    return f4mix.astype(np.float32), grw.astype(np.float32)


# revision 26
# speedup vs baseline: 1.1486x; 1.1486x over previous
"""Trainium2 Bass kernel for nn_Box3DHeuristic (box-3D flavor transformation
heuristic).

Math (per sample b, D=800 quadrature directions):
  4 species moments (N, F) -> fluxfac -> Z (closure) -> maxwellian-ish
  angular distributions g_s(d) = exp(Vaug(d) . Aaug_s(b)) with
  Aaug = [Z*Fx/|F|, Z*Fy/|F|, Z*Fz/|F|, ln(N/(4 pi) * z/sinh z)],
  delta = g0-g1-g2+g3, masked/weighted angular sums -> coeffs -> mixed
  moments + growth rate.

Layout: directions on SBUF partitions (7 chunks of 128, padded 800->896),
samples on the free dim (tiles of 512). Per (chunk, tile):
  PE : 4 arg matmuls (fp32r, K=4) + 10 bf16 reduction matmuls
       (Stot/Spos per species via tile_position col-groups, Ip/Im)
  ACT: 4 exp (psum -> bf16 sbuf) + relu(delta)
  DVE: relu(-delta), 4x fused (delta>=0)*g
  POOL: the 3-op delta chain
Reductions accumulate in PSUM over the 7 chunks; accumulators are DMA'd to
DRAM scratch and gathered back transposed (samples on partitions) for the
scalar postprocессing, which is done batched for the whole core.

8 cores, pure data parallel over the 32768-sample batch.
"""

import math
import sys

sys.path.insert(0, "/opt/trn_rl_repo")

import numpy as np
import ml_dtypes

import concourse.bacc as bacc
import concourse.mybir as mybir
from concourse import tile
from concourse.bass_utils import run_bass_kernel_spmd

F32 = mybir.dt.float32
F32R = mybir.dt.float32r
BF16 = mybir.dt.bfloat16
AF = mybir.ActivationFunctionType
ALU = mybir.AluOpType

B_TOTAL = 32768
N_CORES = 8
NPC = B_TOTAL // N_CORES  # samples per core = 4096
NB = 512  # samples per tile
D = 800
DP = 896  # padded dirs
NCH = DP // 128  # 7 chunks
EPS = 1e-12
INV_4PI = 1.0 / (4.0 * math.pi)
LN2 = math.log(2.0)


def _build(npc=NPC):
    """Build the per-core Bass program (SPMD across 8 cores)."""
    nt = npc // NB  # sample tiles
    npb = npc // 128  # partition blocks of samples
    pb_per_tile = NB // 128  # 4

    nc = bacc.Bacc("TRN2", target_bir_lowering=False, debug=False)

    f4 = nc.declare_dram_parameter("f4", [npc, 24], F32, isOutput=False)
    # block-diagonal Vaug: row s*4+k of the s-block is Vaug[k], else 0 --
    # lets the arg matmul consume the full [16, NB] transposed-A tile
    # without 4-partition slicing (engines need 32-aligned bases)
    vaugbd = nc.declare_dram_parameter("vaugbd", [16, 4 * DP], F32R, isOutput=False)
    wq = nc.declare_dram_parameter("wq", [128, 4 * NCH], BF16, isOutput=False)
    ident = nc.declare_dram_parameter("ident", [128, 128], F32, isOutput=False)
    outf4 = nc.declare_dram_parameter("outf4", [npc, 24], F32, isOutput=True)
    growth = nc.declare_dram_parameter("growth", [npc], F32, isOutput=True)

    # DRAM scratch for the accumulator re-layout (transpose via DMA),
    # rows = 4s+q packed
    stot_d = nc.dram_tensor("stot_d", [16, npc], F32)
    spos_d = nc.dram_tensor("spos_d", [16, npc], F32)
    ipim_d = nc.dram_tensor("ipim_d", [2, npc], F32)

    with tile.TileContext(nc) as tc:
        with (
            tc.tile_pool(name="cst", bufs=1) as cst,
            tc.tile_pool(name="pre", bufs=1) as pre,
            tc.tile_pool(name="sb", bufs=2) as sb,
            tc.tile_pool(name="gp", bufs=2) as gp,
            tc.tile_pool(name="post", bufs=1) as post,
            tc.tile_pool(name="ps_work", bufs=3, space="PSUM") as ps_work,
            tc.tile_pool(name="ps_rhs", bufs=2, space="PSUM") as ps_rhs_pool,
            tc.tile_pool(name="ps_acc", bufs=1, space="PSUM") as ps_acc,
        ):
            # ---- constants in ----
            v_sb = cst.tile([16, 4 * DP], F32R, tag="v")
            nc.sync.dma_start(v_sb[:], vaugbd[:])
            wq_sb = cst.tile([128, 4 * NCH], BF16, tag="wq")
            nc.sync.dma_start(wq_sb[:], wq[:])
            id_sb = cst.tile([128, 128], F32, tag="id")
            nc.sync.dma_start(id_sb[:], ident[:])

            # ---- phase 1: load + preprocess (whole core, batched) ----
            raw = pre.tile([128, npb * 24], F32, tag="raw")
            r3 = raw[:].rearrange("p (t c) -> p t c", c=24)
            nc.sync.dma_start(
                r3, f4[:].rearrange("(t p) c -> p t c", p=128)
            )
            tc.strict_bb_all_engine_barrier()

            # species moments  spec[p, t, s, c], c = [Fx, Fy, Fz, N]
            spec = pre.tile([128, npb * 16], F32, tag="spec")
            s4 = spec[:].rearrange("p (t s c) -> p t s c", s=4, c=4)
            nc.vector.tensor_copy(s4[:, :, 0, :], r3[:, :, 0:4])
            nc.vector.tensor_copy(s4[:, :, 1, :], r3[:, :, 12:16])
            nc.vector.tensor_add(s4[:, :, 2, :], r3[:, :, 4:8], r3[:, :, 8:12])
            nc.vector.tensor_add(s4[:, :, 3, :], r3[:, :, 16:20], r3[:, :, 20:24])

            def qt(tag):
                t = pre.tile([128, npb * 4], F32, tag=tag)
                return t, t[:].rearrange("p (t s) -> p t s", s=4)

            sq = pre.tile([128, npb * 16], F32, tag="sq")
            q4 = sq[:].rearrange("p (t s c) -> p t s c", s=4, c=4)
            nc.vector.tensor_mul(sq[:], spec[:], spec[:])
            n2_t, n2 = qt("n2")
            nc.vector.tensor_add(n2, q4[:, :, :, 0], q4[:, :, :, 1])
            nc.vector.tensor_add(n2, n2, q4[:, :, :, 2])
            nc.vector.tensor_scalar(n2, n2, 1e-24, None, ALU.max)
            lnn2_t, lnn2 = qt("lnn2")
            nc.scalar.activation(lnn2, n2, AF.Ln)
            nrm_t, nrm = qt("nrm")
            nc.scalar.activation(nrm, lnn2, AF.Exp, scale=0.5)
            rn_t, rn = qt("rn")
            nc.scalar.activation(rn, lnn2, AF.Exp, scale=-0.5)
            recn_t, recn = qt("recn")
            nc.vector.reciprocal(recn, s4[:, :, :, 3])
            ff_t, ff = qt("ff")
            nc.vector.tensor_mul(ff, nrm, recn)
            nc.vector.tensor_scalar(ff, ff, 0.999999, None, ALU.min)

            # Z = 2f / max(2*(1-f)*(1+1.01524 f)/den, 1e-6)
            f2_t, f2 = qt("f2")
            nc.vector.tensor_mul(f2, ff, ff)
            f4q_t, f4q = qt("f4q")
            nc.vector.tensor_mul(f4q, f2, f2)
            f6_t, f6 = qt("f6")
            nc.vector.tensor_mul(f6, f4q, f2)
            f8_t, f8 = qt("f8")
            nc.vector.tensor_mul(f8, f4q, f4q)
            den_t, den = qt("den")
            nc.vector.tensor_scalar(den, f2, -1.00651, 3.0, ALU.mult, ALU.add)
            nc.vector.scalar_tensor_tensor(den, f4q, -0.962251, den, ALU.mult, ALU.add)
            nc.vector.scalar_tensor_tensor(den, f6, 1.47353, den, ALU.mult, ALU.add)
            nc.vector.scalar_tensor_tensor(den, f8, -0.48953, den, ALU.mult, ALU.add)
            u_t, uu = qt("u")
            nc.vector.tensor_scalar(uu, ff, 1.01524, 1.0, ALU.mult, ALU.add)
            v_t, vv = qt("vv")
            nc.vector.tensor_scalar(vv, ff, -1.0, 1.0, ALU.mult, ALU.add)
            uv_t, uv = qt("uv")
            nc.vector.tensor_mul(uv, uu, vv)
            rden_t, rden = qt("rden")
            nc.vector.reciprocal(rden, den)
            tq_t, tq = qt("tq")
            nc.vector.tensor_mul(tq, uv, rden)
            zden_t, zden = qt("zden")
            nc.vector.tensor_scalar(zden, tq, 2.0, 1e-6, ALU.mult, ALU.max)
            rzden_t, rzden = qt("rzden")
            nc.vector.reciprocal(rzden, zden)
            zz_t, zz = qt("zz")
            nc.vector.scalar_tensor_tensor(zz, ff, 2.0, rzden, ALU.mult, ALU.mult)

            # w = z/sinh(z) = 2 z e^-z / (1 - e^-2z), series for z < 1e-3
            em_t, em = qt("em")
            nc.scalar.activation(em, zz, AF.Exp, scale=-1.0)
            em2_t, em2 = qt("em2")
            nc.vector.tensor_mul(em2, em, em)
            d2_t, d2 = qt("d2")
            nc.vector.tensor_scalar(d2, em2, -1.0, 1.0, ALU.mult, ALU.add)
            nc.vector.tensor_scalar(d2, d2, 1e-20, None, ALU.max)
            rd2_t, rd2 = qt("rd2")
            nc.vector.reciprocal(rd2, d2)
            num_t, num = qt("num")
            nc.vector.scalar_tensor_tensor(num, zz, 2.0, em, ALU.mult, ALU.mult)
            wzs_t, wzs = qt("wzs")
            nc.vector.tensor_mul(wzs, num, rd2)
            z2_t, z2 = qt("z2")
            nc.vector.tensor_mul(z2, zz, zz)
            z4_t, z4 = qt("z4")
            nc.vector.tensor_mul(z4, z2, z2)
            ser_t, ser = qt("ser")
            nc.vector.tensor_scalar(ser, z2, -1.0 / 6.0, 1.0, ALU.mult, ALU.add)
            nc.vector.scalar_tensor_tensor(ser, z4, 1.0 / 120.0, ser, ALU.mult, ALU.add)
            msmall_t = pre.tile([128, npb * 4], mybir.dt.uint8, tag="msmall")
            msmall = msmall_t[:].rearrange("p (t s) -> p t s", s=4)
            nc.vector.tensor_single_scalar(msmall, zz, 1e-3, ALU.is_lt)
            nc.vector.copy_predicated(wzs, msmall, ser)

            # lnc = ln(N * wzs * INV_4PI); x/xbar species carry -ln2 (their
            # N/F are sums of 2 flavors, not means -- scale cancels
            # everywhere except here)
            prod_t, prod = qt("prod")
            nc.vector.tensor_mul(prod, s4[:, :, :, 3], wzs)
            lnc_t, lnc = qt("lnc")
            nc.scalar.activation(lnc, prod, AF.Ln, scale=INV_4PI)
            nc.vector.tensor_scalar(
                lnc[:, :, 2:4], lnc[:, :, 2:4], -LN2, None, ALU.add
            )

            # pre_A[p, t, s, k] = [Z*Fx/|F|, Z*Fy/|F|, Z*Fz/|F|, lnc]
            zr_t, zr = qt("zr")
            nc.vector.tensor_mul(zr, zz, rn)
            pre_a = pre.tile([128, npb * 16], F32, tag="pre_a")
            a4 = pre_a[:].rearrange("p (t s k) -> p t s k", s=4, k=4)
            for k in range(3):
                nc.vector.tensor_mul(a4[:, :, :, k], s4[:, :, :, k], zr)
            nc.vector.tensor_copy(a4[:, :, :, 3], lnc)

            # ---- phase 2: per sample-tile main loop ----
            for n in range(nt):
                # transpose pre_A block -> rhs [16, 512] (k,s on partitions)
                ps_rhs = ps_rhs_pool.tile([16, NB], F32, tag="ps_rhs")
                for j in range(pb_per_tile):
                    pb = n * pb_per_tile + j
                    nc.tensor.transpose(
                        ps_rhs[:, 128 * j : 128 * (j + 1)],
                        pre_a[:, 16 * pb : 16 * (pb + 1)],
                        id_sb[:],
                    )
                rhs_all = sb.tile([16, NB], F32R, tag="rhs")
                nc.vector.tensor_copy(rhs_all[:], ps_rhs[:])

                acc_tot = ps_acc.tile([128, NB], F32, tag="acc_tot")
                acc_pos = ps_acc.tile([128, NB], F32, tag="acc_pos")
                acc_ipim = ps_acc.tile([128, NB], F32, tag="acc_ipim")

                for c in range(NCH):
                    gs = []
                    for s in range(4):
                        arg = ps_work.tile([128, NB], F32, tag="arg")
                        nc.tensor.matmul(
                            arg[:],
                            v_sb[:, s * DP + 128 * c : s * DP + 128 * (c + 1)],
                            rhs_all[:],
                            start=True,
                            stop=True,
                        )
                        g = gp.tile([128, NB], BF16, tag=f"g{s}")
                        nc.scalar.activation(g[:], arg[:], AF.Exp)
                        gs.append(g)

                    t1 = gp.tile([128, NB], BF16, tag="t1")
                    nc.gpsimd.tensor_sub(t1[:], gs[0][:], gs[1][:])
                    t2 = gp.tile([128, NB], BF16, tag="t2")
                    nc.gpsimd.tensor_sub(t2[:], gs[2][:], gs[3][:])
                    delta = gp.tile([128, NB], BF16, tag="delta")
                    nc.vector.tensor_sub(delta[:], t1[:], t2[:])

                    rpos = gp.tile([128, NB], BF16, tag="rpos")
                    nc.scalar.activation(rpos[:], delta[:], AF.Relu)
                    rneg = gp.tile([128, NB], BF16, tag="rneg")
                    nc.vector.tensor_scalar(
                        rneg[:], delta[:], -1.0, 0.0, ALU.mult, ALU.max
                    )
                    # (delta >= 0) as a bf16 0/1 mask; 1-input TSP is ~2x
                    # faster than the fused 2-tensor-input form
                    maskb = gp.tile([128, NB], BF16, tag="maskb")
                    nc.vector.tensor_single_scalar(
                        maskb[:], delta[:], 0.0, ALU.is_ge
                    )

                    wqc = wq_sb[:, 4 * c : 4 * c + 4]
                    mmkw = dict(start=(c == 0), stop=(c == NCH - 1))
                    for s in range(4):
                        gpos = gp.tile([128, NB], BF16, tag=f"gp{s}")
                        nc.vector.tensor_mul(gpos[:], gs[s][:], maskb[:])
                        nc.tensor.matmul(
                            acc_tot[32 * s : 32 * s + 4, :],
                            wqc,
                            gs[s][:],
                            tile_position=(0, 32 * s),
                            **mmkw,
                        )
                        nc.tensor.matmul(
                            acc_pos[32 * s : 32 * s + 4, :],
                            wqc,
                            gpos[:],
                            tile_position=(0, 32 * s),
                            **mmkw,
                        )
                    nc.tensor.matmul(
                        acc_ipim[0:4, :], wqc, rpos[:], tile_position=(0, 0), **mmkw
                    )
                    nc.tensor.matmul(
                        acc_ipim[32:36, :], wqc, rneg[:], tile_position=(0, 32), **mmkw
                    )

                # PSUM has no DMA route: stage written row-groups through
                # SBUF (engine partition bases must be 32-aligned; the DMAs
                # do the row packing)
                sl = slice(NB * n, NB * (n + 1))
                sb_tot = sb.tile([128, NB], F32, tag="sb_tot")
                sb_pos = sb.tile([128, NB], F32, tag="sb_pos")
                sb_ipim = sb.tile([36, NB], F32, tag="sb_ipim")
                for s in range(4):
                    nc.vector.tensor_copy(
                        sb_tot[32 * s : 32 * s + 4, :], acc_tot[32 * s : 32 * s + 4, :]
                    )
                    nc.scalar.copy(
                        sb_pos[32 * s : 32 * s + 4, :], acc_pos[32 * s : 32 * s + 4, :]
                    )
                    nc.sync.dma_start(
                        stot_d[4 * s : 4 * s + 4, sl], sb_tot[32 * s : 32 * s + 4, :]
                    )
                    nc.sync.dma_start(
                        spos_d[4 * s : 4 * s + 4, sl], sb_pos[32 * s : 32 * s + 4, :]
                    )
                nc.vector.tensor_copy(sb_ipim[0:4, :], acc_ipim[0:4, :])
                nc.vector.tensor_copy(sb_ipim[32:36, :], acc_ipim[32:36, :])
                nc.sync.dma_start(ipim_d[0:1, sl], sb_ipim[3:4, :])
                nc.sync.dma_start(ipim_d[1:2, sl], sb_ipim[35:36, :])

            # ---- phase 3: reload scratch, PE-transpose to sample-major ----
            ld_tot = post.tile([16, npc], F32, tag="ld_tot")
            nc.sync.dma_start(ld_tot[:], stot_d[:])
            ld_pos = post.tile([16, npc], F32, tag="ld_pos")
            nc.sync.dma_start(ld_pos[:], spos_d[:])
            ld_ipim = post.tile([2, npc], F32, tag="ld_ipim")
            nc.sync.dma_start(ld_ipim[:], ipim_d[:])
            tc.strict_bb_all_engine_barrier()

            # psum tiles (reuse the accumulator slots, freed by now)
            ptot_ps = ps_acc.tile([128, NB], F32, tag="acc_tot")
            ppos_ps = ps_acc.tile([128, NB], F32, tag="acc_pos")
            pipim_ps = ps_acc.tile([128, NB], F32, tag="acc_ipim")
            for t in range(npb):
                cs = slice(128 * t, 128 * (t + 1))
                nc.tensor.transpose(
                    ptot_ps[:, 16 * t : 16 * (t + 1)], ld_tot[:, cs], id_sb[0:16, 0:16]
                )
                nc.tensor.transpose(
                    ppos_ps[:, 16 * t : 16 * (t + 1)], ld_pos[:, cs], id_sb[0:16, 0:16]
                )
                nc.tensor.transpose(
                    pipim_ps[:, 2 * t : 2 * (t + 1)], ld_ipim[:, cs], id_sb[0:2, 0:2]
                )

            ptot_sb = post.tile([128, npb * 16], F32, tag="ptot_sb")
            nc.vector.tensor_copy(ptot_sb[:], ptot_ps[:, 0 : npb * 16])
            ppos_sb = post.tile([128, npb * 16], F32, tag="ppos_sb")
            nc.scalar.copy(ppos_sb[:], ppos_ps[:, 0 : npb * 16])
            pipim_sb = post.tile([128, npb * 2], F32, tag="pipim_sb")
            nc.vector.tensor_copy(pipim_sb[:], pipim_ps[:, 0 : npb * 2])
            ptot_t = ptot_sb[:]
            ptot = ptot_t.rearrange("p (t s q) -> p t s q", s=4, q=4)
            ppos_t = ppos_sb[:]
            ip2 = pipim_sb[:].rearrange("p (t i) -> p t i", i=2)

            def pt(tag, w=1):
                t = post.tile([128, npb * w], F32, tag=tag)
                if w == 1:
                    return t, t[:]
                return t, t[:].rearrange("p (t s) -> p t s", s=w)

            ipt, ip = pt("ip")
            nc.vector.tensor_scalar(ip, ip2[:, :, 0], 0.0, None, ALU.max)
            imt, im = pt("im")
            nc.vector.tensor_scalar(im, ip2[:, :, 1], 0.0, None, ALU.max)
            # growth = sqrt(ip*im) via ln/exp
            gprod_t, gprod = pt("gprod")
            nc.vector.tensor_mul(gprod, ip, im)
            nc.vector.tensor_scalar(gprod, gprod, 1e-38, None, ALU.max)
            grw_t, grw = pt("grw")
            nc.scalar.activation(grw, gprod, AF.Ln)
            nc.scalar.activation(grw, grw, AF.Exp, scale=0.5)
            # growth is sample-contiguous in DRAM: PE-transpose first
            grw_ps = ps_work.tile([npb, 128], F32, tag="arg")
            nc.tensor.transpose(grw_ps[:], grw, id_sb[:])
            grw_sb = post.tile([npb, 128], F32, tag="grw_sb")
            nc.vector.tensor_copy(grw_sb[:], grw_ps[:])
            nc.sync.dma_start(
                growth[:].rearrange("(t p) -> t p", p=128), grw_sb[:]
            )

            ips_t, ips = pt("ips")
            nc.vector.tensor_scalar(ips, ip, EPS, None, ALU.max)
            ims_t, ims = pt("ims")
            nc.vector.tensor_scalar(ims, im, EPS, None, ALU.max)
            rip_t, rip = pt("rip")
            nc.vector.reciprocal(rip, ips)
            rim_t, rim = pt("rim")
            nc.vector.reciprocal(rim, ims)
            cp_t, cp = pt("cp")
            nc.vector.scalar_tensor_tensor(cp, ims, -2.0 / 3.0, rip, ALU.mult, ALU.mult)
            nc.vector.tensor_scalar(cp, cp, 1.0, None, ALU.add)
            cn_t, cn = pt("cn")
            nc.vector.scalar_tensor_tensor(cn, ips, -2.0 / 3.0, rim, ALU.mult, ALU.mult)
            nc.vector.tensor_scalar(cn, cn, 1.0, None, ALU.add)
            third_t, third = pt("third")
            nc.vector.memset(third, 1.0 / 3.0)
            clt_t = post.tile([128, npb], mybir.dt.uint8, tag="clt")
            clt = clt_t[:]
            nc.vector.tensor_tensor(clt, ip, im, ALU.is_lt)
            cge_t = post.tile([128, npb], mybir.dt.uint8, tag="cge")
            cge = cge_t[:]
            nc.vector.tensor_tensor(cge, ip, im, ALU.is_ge)
            nc.vector.copy_predicated(cp, clt, third)
            nc.vector.copy_predicated(cn, cge, third)
            nc.vector.tensor_scalar(cp, cp, 0.0, 1.0, ALU.max, ALU.min)
            nc.vector.tensor_scalar(cn, cn, 0.0, 1.0, ALU.max, ALU.min)
            # half coeffs for the assembly
            cnh_t, cnh = pt("cnh")
            nc.vector.tensor_scalar(cnh, cn, 0.5, None, ALU.mult)
            dch_t, dch = pt("dch")
            nc.vector.tensor_sub(dch, cp, cn)
            nc.vector.tensor_scalar(dch, dch, 0.5, None, ALU.mult)

            # Uh[p,t,s,q] = 0.5 * (cn*Stot + (cp-cn)*Spos)
            uh = post.tile([128, npb * 16], F32, tag="uh")
            u4 = uh[:].rearrange("p (t s q) -> p t s q", s=4, q=4)
            tmp16 = post.tile([128, 16], F32, tag="tmp16")
            for t in range(npb):
                nc.vector.tensor_scalar(
                    tmp16[:], ppos_t[:, 16 * t : 16 * (t + 1)],
                    dch_t[:, t : t + 1], None, ALU.mult,
                )
                nc.vector.scalar_tensor_tensor(
                    uh[:, 16 * t : 16 * (t + 1)],
                    ptot_t[:, 16 * t : 16 * (t + 1)],
                    cnh_t[:, t : t + 1],
                    tmp16[:],
                    ALU.mult,
                    ALU.add,
                )

            # output species sums:
            # out0 = 2*Uh0 + S2 - 2*Uh2        (nue_t)
            # out1 = 2*Uh1 + S3 - 2*Uh3        (nuebar_t)
            # out2 = 0.5*(S0+S2) + Uh2 - Uh0   (nux_t)
            # out3 = 0.5*(S1+S3) + Uh3 - Uh1   (nuxbar_t)
            newq = post.tile([128, npb * 16], F32, tag="newq")
            n4 = newq[:].rearrange("p (t s q) -> p t s q", s=4, q=4)
            ta = post.tile([128, npb * 4], F32, tag="ta")
            ta3 = ta[:].rearrange("p (t q) -> p t q", q=4)
            tb = post.tile([128, npb * 4], F32, tag="tb")
            tb3 = tb[:].rearrange("p (t q) -> p t q", q=4)
            nc.vector.scalar_tensor_tensor(
                ta3, u4[:, :, 2, :], -2.0, ptot[:, :, 2, :], ALU.mult, ALU.add
            )
            nc.vector.scalar_tensor_tensor(
                n4[:, :, 0, :], u4[:, :, 0, :], 2.0, ta3, ALU.mult, ALU.add
            )
            nc.vector.scalar_tensor_tensor(
                ta3, u4[:, :, 3, :], -2.0, ptot[:, :, 3, :], ALU.mult, ALU.add
            )
            nc.vector.scalar_tensor_tensor(
                n4[:, :, 1, :], u4[:, :, 1, :], 2.0, ta3, ALU.mult, ALU.add
            )
            nc.vector.tensor_add(ta3, ptot[:, :, 0, :], ptot[:, :, 2, :])
            nc.vector.tensor_sub(tb3, u4[:, :, 2, :], u4[:, :, 0, :])
            nc.vector.scalar_tensor_tensor(
                n4[:, :, 2, :], ta3, 0.5, tb3, ALU.mult, ALU.add
            )
            nc.vector.tensor_add(ta3, ptot[:, :, 1, :], ptot[:, :, 3, :])
            nc.vector.tensor_sub(tb3, u4[:, :, 3, :], u4[:, :, 1, :])
            nc.vector.scalar_tensor_tensor(
                n4[:, :, 3, :], ta3, 0.5, tb3, ALU.mult, ALU.add
            )

            # assemble [p, t, 6 slots, 4] and store
            osb = post.tile([128, npb * 24], F32, tag="osb")
            o4 = osb[:].rearrange("p (t s c) -> p t s c", s=6, c=4)
            for slot, src in ((0, 0), (1, 2), (2, 2), (3, 1), (4, 3), (5, 3)):
                nc.vector.tensor_copy(o4[:, :, slot, :], n4[:, :, src, :])
            nc.sync.dma_start(
                outf4[:].rearrange("(t p) c -> p t c", p=128),
                osb[:].rearrange("p (t c) -> p t c", c=24),
            )

    nc.compile()
    return nc


def _make_consts(dir_x, dir_y, dir_z, quad_w):
    """Build block-diag Vaug [16, 4*DP] (f32) and wq [128, 4*NCH] (bf16)."""
    vx = np.zeros(DP, np.float32)
    vy = np.zeros(DP, np.float32)
    vz = np.zeros(DP, np.float32)
    w = np.zeros(DP, np.float32)
    vx[:D], vy[:D], vz[:D], w[:D] = dir_x, dir_y, dir_z, quad_w
    vaug = np.stack([vx, vy, vz, np.ones(DP, np.float32)])  # [4, DP]
    vaugbd = np.zeros((16, 4 * DP), np.float32)
    for s in range(4):
        vaugbd[4 * s : 4 * s + 4, s * DP : (s + 1) * DP] = vaug
    # wq[p, 4c+j]: dir d = 128c + p, cols [w*vx, w*vy, w*vz, w]
    cols = np.stack([w * vx, w * vy, w * vz, w], axis=1)  # [DP, 4]
    wq = cols.reshape(NCH, 128, 4).transpose(1, 0, 2).reshape(128, 4 * NCH)
    return vaugbd, wq.astype(ml_dtypes.bfloat16)


_NC_CACHE = {}
TRACE = False  # set True (e.g. from test.py) to capture an NTFF profile
LAST_RESULTS = None  # BassKernelResults of the most recent kernel() call


def kernel(F4_in, dir_x, dir_y, dir_z, quad_w):
    F4_in = np.asarray(F4_in, np.float32)
    dir_x = np.asarray(dir_x, np.float32)
    dir_y = np.asarray(dir_y, np.float32)
    dir_z = np.asarray(dir_z, np.float32)
    quad_w = np.asarray(quad_w, np.float32)
    b = F4_in.shape[0]
    npc = b // N_CORES
    if npc not in _NC_CACHE:
        _NC_CACHE[npc] = _build(npc)
    nc = _NC_CACHE[npc]

    vaugbd, wq = _make_consts(dir_x, dir_y, dir_z, quad_w)
    ident = np.eye(128, dtype=np.float32)
    flat = np.ascontiguousarray(F4_in.reshape(b, 24))
    in_maps = [
        dict(
            f4=flat[i * npc : (i + 1) * npc],
            vaugbd=vaugbd,
            wq=wq,
            ident=ident,
        )
        for i in range(N_CORES)
    ]
    res = run_bass_kernel_spmd(
        nc, in_maps, core_ids=list(range(N_CORES)), trace=TRACE
    )
    global LAST_RESULTS
    LAST_RESULTS = res
    f4mix = np.concatenate([r["outf4"] for r in res.results]).reshape(b, 2, 3, 4)
    grw = np.concatenate([r["growth"] for r in res.results])
    return f4mix.astype(np.float32), grw.astype(np.float32)
